# revision 9
# baseline (speedup 1.0000x reference)
"""Causal attention (B=4, S=2048, D=1024) on 8 TRN2 NeuronCores.

Sharding: core c -> batch c//2, query-half c%2. Each core computes K/V for
all 2048 keys of its batch and attention for 1024 queries. Queries are
regrouped (host-side) into 4 groups of 256 pairing complementary causal
blocks, so one SPMD program with a fixed key-prefix schedule [4,8,12,16]
kblocks serves both halves; per-core causal structure lives in input data
(xqT column gather + qpos vector), never in program constants.

Math: scoresT[k,q] = KT^T QT accumulated over d in PSUM, probs =
exp(scoresT/32) (no max subtraction: logits ~ N(0,1)), causal mask applied
as a multiplicative (qpos >= kpos) keep-mask after exp, out = P^T V with
row-sums from a ones-column matmul, normalized at eviction.

Precision: Q/K/scores in float32r (tf32-class), V/probs bf16, fp32 accum.

DMA is spread across engine queues (sync: x, scalar: weights, gpsimd:
v_tmp/out writes, vector: v reads) to avoid single-queue serialization.
"""

import numpy as np

import concourse.bass as bass
import concourse.mybir as mybir
import concourse.tile as tile
from concourse import bacc
from concourse.bass_utils import run_bass_kernel_spmd

B, S, D = 4, 2048, 1024
P = 128
NQ = S // 2               # queries per core
DT = D // P               # 8 d-tiles
KI = D // P               # 8 contraction tiles
NKB = S // P              # 16 key blocks
NG = 4                    # query groups per core
GQ = 256                  # queries per group
LKB = [4, 8, 12, 16]      # key-prefix (in kblocks) per group
MASK_START = [0, 4, 8, 12]  # first kblock needing the causal keep-mask

# per-core query block order (global block index within the batch)
QLIST = {
    0: [0, 2, 4, 6, 9, 11, 13, 15],
    1: [1, 3, 5, 7, 8, 10, 12, 14],
}

F32 = mybir.dt.float32
F32R = mybir.dt.float32r
BF16 = mybir.dt.bfloat16
AF = mybir.ActivationFunctionType

_NC_CACHE = []


def _build_nc():
    nc = bacc.Bacc("TRN2")
    xT = nc.dram_tensor("xT", [D, S], F32, kind="ExternalInput")
    xqT = nc.dram_tensor("xqT", [D, NQ], F32, kind="ExternalInput")
    # weights host-blocked: [DT, P, KI*P] with [dt, p, ki, o] = WT[ki*P+p, dt*P+o]
    wq = nc.dram_tensor("wq", [DT, P, KI * P], F32, kind="ExternalInput")
    wk = nc.dram_tensor("wk", [DT, P, KI * P], F32, kind="ExternalInput")
    # wv blocked: [2, P, KI*512] with [oc, p, ki, o] = WvT[ki*P+p, oc*512+o]
    wv = nc.dram_tensor("wv", [2, P, KI * 512], F32, kind="ExternalInput")
    qpos = nc.dram_tensor("qpos", [1, NQ], F32, kind="ExternalInput")
    out = nc.dram_tensor("out", [NQ, D], F32, kind="ExternalOutput")
    v_tmp = nc.dram_tensor("v_tmp", [NKB, P, D], BF16, kind="Internal")

    with tile.TileContext(nc) as tc:
        with tc.tile_pool(name="const", bufs=1) as const:
            ones_row = const.tile([1, P], F32, name="ones_row")
            nc.vector.memset(ones_row[:], 1.0)
            ones_col = const.tile([P, 1], BF16, name="ones_col")
            nc.vector.memset(ones_col[:], 1.0)
            kpos_i = const.tile([P, 1], mybir.dt.int32, name="kpos_i")
            nc.gpsimd.iota(kpos_i[:], pattern=[[0, 1]], base=0, channel_multiplier=1)
            kpos_f = const.tile([P, 1], F32, name="kpos_f")
            nc.vector.tensor_copy(kpos_f[:], kpos_i[:])
            qpos_sb = const.tile([1, NQ], F32, name="qpos_sb")
            nc.scalar.dma_start(out=qpos_sb[:], in_=qpos[:])

            with tc.tile_pool(name="qt_p", bufs=1) as qt_pool, \
                 tc.tile_pool(name="xtr_p", bufs=8) as xtr_pool:
                qt_sb = qt_pool.tile([P, DT, NQ], F32R, name="qt_sb")
                xtr = []

                # ---- Q projection, with xT load/round interleaved ----
                with tc.tile_pool(name="xqr_p", bufs=8) as xqr_pool, \
                     tc.tile_pool(name="xq_st", bufs=2) as xqstage, \
                     tc.tile_pool(name="x_st", bufs=2) as xstage, \
                     tc.tile_pool(name="wq_s", bufs=2) as wq_stage, \
                     tc.tile_pool(name="wq_r", bufs=2) as wq_round, \
                     tc.tile_pool(name="pj_q", bufs=8, space="PSUM") as pj:
                    xqr = []
                    for ki in range(KI):
                        stg = xqstage.tile([P, NQ], F32, name=f"xqs{ki}", tag="xqs")
                        nc.sync.dma_start(
                            out=stg[:], in_=xqT[ki * P:(ki + 1) * P, :]
                        )
                        t = xqr_pool.tile([P, NQ], F32R, name=f"xqr{ki}", tag="xqr")
                        nc.vector.tensor_copy(t[:], stg[:])
                        xqr.append(t)
                    for dt in range(DT):
                        # interleave one xT column-block load per dt iteration
                        stg = xstage.tile([P, S], F32, name=f"xs{dt}", tag="xs")
                        nc.sync.dma_start(
                            out=stg[:], in_=xT[dt * P:(dt + 1) * P, :]
                        )
                        t = xtr_pool.tile([P, S], F32R, name=f"xtr{dt}", tag="xtr")
                        nc.vector.tensor_copy(t[:], stg[:])
                        xtr.append(t)

                        wq_st = wq_stage.tile([P, KI * P], F32, name=f"wqs{dt}",
                                              tag="wqs")
                        nc.scalar.dma_start(out=wq_st[:], in_=wq[dt, :, :])
                        wq_r = wq_round.tile([P, KI * P], F32R, name=f"wqr{dt}",
                                             tag="wqr")
                        nc.vector.tensor_copy(wq_r[:], wq_st[:])
                        ps = [
                            pj.tile([P, 512], F32, name=f"psq{dt}_{qc}", tag="pjq")
                            for qc in range(2)
                        ]
                        for ki in range(KI):
                            for qc in range(2):
                                nc.tensor.matmul(
                                    ps[qc][:],
                                    wq_r[:, ki * P:(ki + 1) * P],
                                    xqr[ki][:, qc * 512:(qc + 1) * 512],
                                    start=(ki == 0),
                                    stop=(ki == KI - 1),
                                )
                        for qc in range(2):
                            nc.scalar.copy(
                                qt_sb[:, dt, qc * 512:(qc + 1) * 512], ps[qc][:]
                            )

                # ---- K projection ----
                with tc.tile_pool(name="kt_p", bufs=1) as kt_pool:
                    kt_sb = kt_pool.tile([P, DT, S], F32R, name="kt_sb")
                    with tc.tile_pool(name="wk_s", bufs=2) as wk_stage, \
                         tc.tile_pool(name="wk_r", bufs=2) as wk_round, \
                         tc.tile_pool(name="pj_k", bufs=8, space="PSUM") as pjk:
                        for dt in range(DT):
                            wk_st = wk_stage.tile([P, KI * P], F32, name=f"wks{dt}",
                                                  tag="wks")
                            nc.scalar.dma_start(out=wk_st[:], in_=wk[dt, :, :])
                            wk_r = wk_round.tile([P, KI * P], F32R, name=f"wkr{dt}",
                                                 tag="wkr")
                            nc.vector.tensor_copy(wk_r[:], wk_st[:])
                            ps = [
                                pjk.tile([P, 512], F32, name=f"psk{dt}_{kc}",
                                         tag="pjk")
                                for kc in range(4)
                            ]
                            for ki in range(KI):
                                for kc in range(4):
                                    nc.tensor.matmul(
                                        ps[kc][:],
                                        wk_r[:, ki * P:(ki + 1) * P],
                                        xtr[ki][:, kc * 512:(kc + 1) * 512],
                                        start=(ki == 0),
                                        stop=(ki == KI - 1),
                                    )
                            for kc in range(4):
                                nc.scalar.copy(
                                    kt_sb[:, dt, kc * 512:(kc + 1) * 512], ps[kc][:]
                                )

                    # ---- V projection -> DRAM (bf16) ----
                    with tc.tile_pool(name="wv_s", bufs=2) as wv_stage, \
                         tc.tile_pool(name="wv_r", bufs=8) as wv_round, \
                         tc.tile_pool(name="v_ev", bufs=3) as vst_pool, \
                         tc.tile_pool(name="pj_v", bufs=8, space="PSUM") as pjv:
                        for oc in range(2):
                            wv_r = []
                            for ki in range(KI):
                                stg = wv_stage.tile([P, 512], F32,
                                                    name=f"wvs{oc}_{ki}", tag="wvs")
                                nc.scalar.dma_start(
                                    out=stg[:],
                                    in_=wv[oc, :, ki * 512:(ki + 1) * 512],
                                )
                                t = wv_round.tile([P, 512], F32R,
                                                  name=f"wvr{oc}_{ki}", tag="wvr")
                                nc.vector.tensor_copy(t[:], stg[:])
                                wv_r.append(t)
                            for st_i in range(NKB):
                                ps = pjv.tile([P, 512], F32, name=f"psv{oc}_{st_i}",
                                              tag="pjv")
                                for ki in range(KI):
                                    nc.tensor.matmul(
                                        ps[:],
                                        xtr[ki][:, st_i * P:(st_i + 1) * P],
                                        wv_r[ki][:],
                                        start=(ki == 0),
                                        stop=(ki == KI - 1),
                                    )
                                vs = vst_pool.tile([P, 512], BF16,
                                                   name=f"vsb{oc}_{st_i}", tag="vsb")
                                nc.scalar.copy(vs[:], ps[:])
                                nc.gpsimd.dma_start(
                                    out=v_tmp[st_i, :, oc * 512:(oc + 1) * 512],
                                    in_=vs[:],
                                )

                    # ---- attention ----
                    with tc.tile_pool(name="att", bufs=2) as att, \
                         tc.tile_pool(name="vin", bufs=3) as vin, \
                         tc.tile_pool(name="ptp", bufs=3) as ptp, \
                         tc.tile_pool(name="scp", bufs=2, space="PSUM") as scp, \
                         tc.tile_pool(name="avp", bufs=4, space="PSUM") as avp, \
                         tc.tile_pool(name="smp", bufs=1, space="PSUM") as smp, \
                         tc.tile_pool(name="evp", bufs=2) as ev:
                        for g in range(NG):
                            lkb = LKB[g]
                            ps_bc = scp.tile([P, GQ], F32, name=f"psbc{g}", tag="sc")
                            nc.tensor.matmul(
                                ps_bc[:],
                                ones_row[:],
                                qpos_sb[:, g * GQ:(g + 1) * GQ],
                                start=True,
                                stop=True,
                            )
                            qbc = att.tile([P, GQ], F32, name=f"qbc{g}", tag="qbc")
                            nc.vector.tensor_copy(qbc[:], ps_bc[:])

                            oa = [
                                avp.tile([P, 512], F32, name=f"oa{g}_{i}", tag="av")
                                for i in range(4)
                            ]
                            sm = [
                                smp.tile([P, 1], F32, name=f"sm{g}_{i}",
                                         tag=f"sm{i}")
                                for i in range(2)
                            ]

                            for kb in range(lkb):
                                ps_s = scp.tile([P, GQ], F32, name=f"pss{g}_{kb}",
                                                tag="sc")
                                for di in range(DT):
                                    nc.tensor.matmul(
                                        ps_s[:],
                                        kt_sb[:, di, kb * P:(kb + 1) * P],
                                        qt_sb[:, di, g * GQ:(g + 1) * GQ],
                                        start=(di == 0),
                                        stop=(di == DT - 1),
                                    )
                                pt = ptp.tile([P, GQ], BF16, name=f"pt{g}_{kb}",
                                              tag="pt")
                                nc.scalar.activation(
                                    pt[:], ps_s[:], AF.Exp, bias=0.0,
                                    scale=1.0 / 32.0,
                                )
                                if kb >= MASK_START[g]:
                                    keep = ptp.tile([P, GQ], BF16,
                                                    name=f"kept{g}_{kb}", tag="keep")
                                    if kb == 0:
                                        kp = kpos_f
                                    else:
                                        kp = ptp.tile([P, 1], F32,
                                                      name=f"kpt{g}_{kb}", tag="kp")
                                        nc.vector.tensor_scalar(
                                            kp[:], kpos_f[:], float(kb * P), None,
                                            mybir.AluOpType.add,
                                        )
                                    nc.vector.tensor_scalar(
                                        keep[:], qbc[:], kp[:], None,
                                        mybir.AluOpType.is_ge,
                                    )
                                    nc.vector.tensor_mul(pt[:], pt[:], keep[:])
                                v_sb = vin.tile([P, D], BF16, name=f"vsa{g}_{kb}",
                                                tag="vin")
                                nc.scalar.dma_start(out=v_sb[:], in_=v_tmp[kb, :, :])
                                for tq in range(2):
                                    ptq = pt[:, tq * P:(tq + 1) * P]
                                    first = (kb == 0)
                                    last = (kb == lkb - 1)
                                    nc.tensor.matmul(
                                        oa[2 * tq][:], ptq, v_sb[:, 0:512],
                                        start=first, stop=last,
                                    )
                                    nc.tensor.matmul(
                                        oa[2 * tq + 1][:], ptq, v_sb[:, 512:1024],
                                        start=first, stop=last,
                                    )
                                    nc.tensor.matmul(
                                        sm[tq][:], ptq, ones_col[:],
                                        start=first, stop=last,
                                    )

                            for tq in range(2):
                                recip = ev.tile([P, 1], F32, name=f"rc{g}_{tq}",
                                                tag="recip")
                                nc.vector.reciprocal(recip[:], sm[tq][:])
                                o_sb = ev.tile([P, D], F32, name=f"ob{g}_{tq}",
                                               tag="osb")
                                nc.scalar.mul(
                                    o_sb[:, 0:512], oa[2 * tq][:], recip[:]
                                )
                                nc.vector.tensor_scalar(
                                    o_sb[:, 512:1024], oa[2 * tq + 1][:],
                                    recip[:], None, mybir.AluOpType.mult,
                                )
                                q_local = 2 * g + tq
                                nc.gpsimd.dma_start(
                                    out=out[q_local * P:(q_local + 1) * P, :],
                                    in_=o_sb[:],
                                )
    nc.compile()
    return nc


def get_nc():
    if not _NC_CACHE:
        _NC_CACHE.append(_build_nc())
    return _NC_CACHE[0]


def _block_w(wT, cols):
    # [D, D] -> [D//cols? ...] block layout: [nb, P, KI*cols] with
    # [b, p, ki, o] = wT[ki*P + p, b*cols + o]
    nb = D // cols
    return np.ascontiguousarray(
        wT.reshape(KI, P, nb, cols).transpose(2, 1, 0, 3).reshape(nb, P, KI * cols)
    )


def make_in_maps(x, Wq, Wk, Wv):
    x = np.asarray(x, dtype=np.float32)
    wqb = _block_w(np.asarray(Wq, np.float32).T, P)
    wkb = _block_w(np.asarray(Wk, np.float32).T, P)
    wvb = _block_w(np.asarray(Wv, np.float32).T, 512)
    in_maps = []
    for c in range(8):
        b, h = divmod(c, 2)
        qrows = np.concatenate(
            [np.arange(qb * P, (qb + 1) * P) for qb in QLIST[h]]
        )
        xb = x[b]  # [S, D]
        in_maps.append({
            "xT": np.ascontiguousarray(xb.T),
            "xqT": np.ascontiguousarray(xb[qrows].T),
            "wq": wqb,
            "wk": wkb,
            "wv": wvb,
            "qpos": qrows.astype(np.float32)[None, :],
        })
    return in_maps


def assemble_output(results):
    out = np.empty((B, S, D), dtype=np.float32)
    for c in range(8):
        b, h = divmod(c, 2)
        oc = results[c]["out"]
        for i, qb in enumerate(QLIST[h]):
            out[b, qb * P:(qb + 1) * P, :] = oc[i * P:(i + 1) * P, :]
    return out


def kernel(x, Wq, Wk, Wv):
    nc = get_nc()
    in_maps = make_in_maps(x, Wq, Wk, Wv)
    res = run_bass_kernel_spmd(nc, in_maps, core_ids=list(range(8)), trace=False)
    return assemble_output(res.results)


# revision 19
# speedup vs baseline: 1.2333x; 1.2333x over previous
"""Causal attention (B=4, S=2048, D=1024) on 8 TRN2 NeuronCores.

Sharding: core c -> batch c//2, query-half c%2. Each core computes K/V for
all 2048 keys of its batch and attention for 1024 queries. Queries are
regrouped (host-side) into 4 groups of 256 pairing complementary causal
blocks, so one SPMD program with a fixed key-prefix schedule [4,8,12,16]
kblocks serves both halves; per-core causal structure lives in input data
(xqT column gather + qpos vector), never in program constants.

Math: scoresT[k,q] = KT^T QT accumulated over d in PSUM, probs =
exp(scoresT/32) (no max subtraction: logits ~ N(0,1)), causal mask applied
as a multiplicative (qpos >= kpos) keep-mask after exp, out = P^T V with
row-sums from a ones-column matmul, normalized at eviction.

Precision: Q/K/scores in float32r (tf32-class), V/probs bf16, fp32 accum.

DMA queues are balanced by need-time: sync carries wq0 -> xqT -> wq ->
xT -> wk in arrival order, gpsimd carries wv + v_tmp/out writes, scalar
carries qpos + v_tmp prefetch reads for the AV pass.
"""

from contextlib import ExitStack

import numpy as np

import concourse.bass as bass
import concourse.mybir as mybir
import concourse.tile as tile
from concourse import bacc
from concourse.bass_utils import run_bass_kernel_spmd

B, S, D = 4, 2048, 1024
P = 128
NQ = S // 2               # queries per core
DT = D // P               # 8 d-tiles
KI = D // P               # 8 contraction tiles
NKB = S // P              # 16 key blocks
NG = 4                    # query groups per core
GQ = 256                  # queries per group
LKB = [4, 8, 12, 16]      # key-prefix (in kblocks) per group

# per-core query block order (global block index within the batch)
QLIST = {
    0: [0, 2, 4, 6, 9, 11, 13, 15],
    1: [1, 3, 5, 7, 8, 10, 12, 14],
}

F32 = mybir.dt.float32
F32R = mybir.dt.float32r
BF16 = mybir.dt.bfloat16
AF = mybir.ActivationFunctionType

_NC_CACHE = []


def _build_nc():
    nc = bacc.Bacc("TRN2")
    xT = nc.dram_tensor("xT", [D, S], F32, kind="ExternalInput")
    xqT = nc.dram_tensor("xqT", [D, NQ], F32, kind="ExternalInput")
    # weights host-blocked: [DT, P, KI*P] with [dt, p, ki, o] = WT[ki*P+p, dt*P+o]
    wq = nc.dram_tensor("wq", [DT, P, KI * P], F32, kind="ExternalInput")
    wk = nc.dram_tensor("wk", [DT, P, KI * P], F32, kind="ExternalInput")
    # wv blocked: [2, P, KI*512] with [oc, p, ki, o] = WvT[ki*P+p, oc*512+o]
    wv = nc.dram_tensor("wv", [2, P, KI * 512], F32, kind="ExternalInput")
    qpos = nc.dram_tensor("qpos", [1, NQ], F32, kind="ExternalInput")
    out = nc.dram_tensor("out", [NQ, D], F32, kind="ExternalOutput")
    v_tmp = nc.dram_tensor("v_tmp", [NKB, P, D], BF16, kind="Internal")

    with tile.TileContext(nc) as tc:
        with tc.tile_pool(name="const", bufs=1) as const:
            ones_row = const.tile([1, P], F32, name="ones_row")
            nc.vector.memset(ones_row[:], 1.0)
            ones_col = const.tile([P, 1], BF16, name="ones_col")
            nc.vector.memset(ones_col[:], 1.0)
            kpos_i = const.tile([P, 1], mybir.dt.int32, name="kpos_i")
            nc.gpsimd.iota(kpos_i[:], pattern=[[0, 1]], base=0, channel_multiplier=1)
            kpos_f = const.tile([P, 1], F32, name="kpos_f")
            nc.vector.tensor_copy(kpos_f[:], kpos_i[:])
            qpos_sb = const.tile([1, NQ], F32, name="qpos_sb")
            nc.scalar.dma_start(out=qpos_sb[:], in_=qpos[:])

            # PE warmup: throwaway matmuls during the initial DMA wait so the
            # HAM clock gate opens before real work arrives.
            with tc.tile_pool(name="warm", bufs=1) as warm, \
                 tc.tile_pool(name="warmp", bufs=1, space="PSUM") as warmp:
                wsink = warmp.tile([P, 512], F32, name="wsink")
                wzero = warm.tile([1, 512], F32, name="wzero")
                nc.vector.memset(wzero[:], 0.0)
                for i in range(10):
                    nc.tensor.matmul(wsink[:], ones_row[:], wzero[:],
                                     start=True, stop=True)
                wdrain = warm.tile([P, 1], F32, name="wdrain")
                nc.scalar.copy(wdrain[:], wsink[:, 0:1])

            with tc.tile_pool(name="qt_p", bufs=1) as qt_pool:
                qt_sb = qt_pool.tile([P, DT, NQ], F32R, name="qt_sb")
                xtr_stack = ExitStack()
                xtr_pool = xtr_stack.enter_context(
                    tc.tile_pool(name="xtr_p", bufs=8, side="right")
                )
                xtr = []

                # ---- Q projection ----
                # sync-queue emission order = arrival order: wq0, then xq
                # (interleaved with wq1..3), then remaining wq, then xT, wk.
                with tc.tile_pool(name="xqr_p", bufs=8) as xqr_pool, \
                     tc.tile_pool(name="xq_st", bufs=2) as xqstage, \
                     tc.tile_pool(name="x_st", bufs=2) as xstage, \
                     tc.tile_pool(name="wq_s", bufs=3) as wq_stage, \
                     tc.tile_pool(name="wq_r", bufs=3) as wq_round, \
                     tc.tile_pool(name="pj_q", bufs=8, space="PSUM") as pj:
                    wq_rs = {}

                    def load_wq(dt):
                        stw = wq_stage.tile([P, KI * P], F32, name=f"wqs{dt}",
                                            tag="wqs")
                        nc.sync.dma_start(out=stw[:], in_=wq[dt, :, :])
                        wq_rs[dt] = wq_round.tile([P, KI * P], F32R,
                                                  name=f"wqr{dt}", tag="wqr")
                        nc.vector.tensor_copy(wq_rs[dt][:], stw[:])

                    load_wq(0)
                    xqr = []
                    for ki in range(KI):
                        stg = xqstage.tile([P, NQ], F32, name=f"xqs{ki}", tag="xqs")
                        nc.sync.dma_start(
                            out=stg[:], in_=xqT[ki * P:(ki + 1) * P, :]
                        )
                        t = xqr_pool.tile([P, NQ], F32R, name=f"xqr{ki}", tag="xqr")
                        nc.vector.tensor_copy(t[:], stg[:])
                        xqr.append(t)
                        if ki in (2, 4, 6):
                            load_wq(ki // 2)
                    for dt in range(DT):
                        if dt + 4 < DT:
                            load_wq(dt + 4)
                        wq_r = wq_rs[dt]
                        # one xT column-block load per dt iteration
                        stg = xstage.tile([P, S], F32, name=f"xs{dt}", tag="xs")
                        nc.sync.dma_start(
                            out=stg[:], in_=xT[dt * P:(dt + 1) * P, :]
                        )
                        t = xtr_pool.tile([P, S], F32R, name=f"xtr{dt}", tag="xtr")
                        nc.vector.tensor_copy(t[:], stg[:])
                        xtr.append(t)
                        ps = [
                            pj.tile([P, 512], F32, name=f"psq{dt}_{qc}", tag="pjq")
                            for qc in range(2)
                        ]
                        for ki in range(KI):
                            for qc in range(2):
                                nc.tensor.matmul(
                                    ps[qc][:],
                                    wq_r[:, ki * P:(ki + 1) * P],
                                    xqr[ki][:, qc * 512:(qc + 1) * 512],
                                    start=(ki == 0),
                                    stop=(ki == KI - 1),
                                )
                        for qc in range(2):
                            nc.scalar.copy(
                                qt_sb[:, dt, qc * 512:(qc + 1) * 512], ps[qc][:]
                            )

                # ---- K projection ----
                with tc.tile_pool(name="kt_p", bufs=1) as kt_pool:
                    kt_sb = kt_pool.tile([P, DT, S], F32R, name="kt_sb")
                    with tc.tile_pool(name="wk_s", bufs=3) as wk_stage, \
                         tc.tile_pool(name="wk_r", bufs=2) as wk_round, \
                         tc.tile_pool(name="pj_k", bufs=8, space="PSUM") as pjk:
                        for dt in range(DT):
                            wk_st = wk_stage.tile([P, KI * P], F32, name=f"wks{dt}",
                                                  tag="wks")
                            nc.sync.dma_start(out=wk_st[:], in_=wk[dt, :, :])
                            wk_r = wk_round.tile([P, KI * P], F32R, name=f"wkr{dt}",
                                                 tag="wkr")
                            nc.vector.tensor_copy(wk_r[:], wk_st[:])
                            ps = [
                                pjk.tile([P, 512], F32, name=f"psk{dt}_{kc}",
                                         tag="pjk")
                                for kc in range(4)
                            ]
                            for ki in range(KI):
                                for kc in range(4):
                                    nc.tensor.matmul(
                                        ps[kc][:],
                                        wk_r[:, ki * P:(ki + 1) * P],
                                        xtr[ki][:, kc * 512:(kc + 1) * 512],
                                        start=(ki == 0),
                                        stop=(ki == KI - 1),
                                    )
                            for kc in range(4):
                                nc.scalar.copy(
                                    kt_sb[:, dt, kc * 512:(kc + 1) * 512], ps[kc][:]
                                )

                    # ---- V projection -> DRAM (bf16) ----
                    with tc.tile_pool(name="wv_s", bufs=2) as wv_stage, \
                         tc.tile_pool(name="wv_r", bufs=8) as wv_round, \
                         tc.tile_pool(name="v_ev", bufs=3) as vst_pool, \
                         tc.tile_pool(name="pj_v", bufs=8, space="PSUM") as pjv:
                        for oc in range(2):
                            wv_r = []
                            for ki in range(KI):
                                stg = wv_stage.tile([P, 512], F32,
                                                    name=f"wvs{oc}_{ki}", tag="wvs")
                                nc.gpsimd.dma_start(
                                    out=stg[:],
                                    in_=wv[oc, :, ki * 512:(ki + 1) * 512],
                                )
                                t = wv_round.tile([P, 512], F32R,
                                                  name=f"wvr{oc}_{ki}", tag="wvr")
                                nc.vector.tensor_copy(t[:], stg[:])
                                wv_r.append(t)
                            for st_i in range(NKB):
                                ps = pjv.tile([P, 512], F32, name=f"psv{oc}_{st_i}",
                                              tag="pjv")
                                for ki in range(KI):
                                    nc.tensor.matmul(
                                        ps[:],
                                        xtr[ki][:, st_i * P:(st_i + 1) * P],
                                        wv_r[ki][:],
                                        start=(ki == 0),
                                        stop=(ki == KI - 1),
                                    )
                                vs = vst_pool.tile([P, 512], BF16,
                                                   name=f"vsb{oc}_{st_i}", tag="vsb")
                                nc.scalar.copy(vs[:], ps[:])
                                nc.gpsimd.dma_start(
                                    out=v_tmp[st_i, :, oc * 512:(oc + 1) * 512],
                                    in_=vs[:],
                                )

                    xtr_stack.close()  # free xT (64KB/p) before attention

                    # ---- attention pass 1: all probs tiles, kb-major ----
                    # scoresT for kblock kb covers queries [qmin:1024] where
                    # qmin = (kb//4)*256 (groups below are causally done).
                    # V blocks prefetch into SBUF so the AV pass is DMA-free.
                    pt_tiles = []
                    vcache = []
                    with tc.tile_pool(name="ptp", bufs=16) as ptp, \
                         tc.tile_pool(name="vcp", bufs=16) as vcp:
                      with tc.tile_pool(name="att", bufs=2) as att, \
                           tc.tile_pool(name="scp", bufs=3, space="PSUM") as scp, \
                           tc.tile_pool(name="bcp", bufs=2, space="PSUM") as bcp:
                        qbc = None
                        for kb in range(NKB):
                            v_sb = vcp.tile([P, D], BF16, name=f"vc{kb}", tag="vc")
                            nc.scalar.dma_start(out=v_sb[:], in_=v_tmp[kb, :, :])
                            vcache.append(v_sb)

                            gmin = kb // 4
                            qmin = gmin * GQ
                            width = NQ - qmin
                            if kb % 4 == 0:
                                ps_bc = bcp.tile([P, GQ], F32, name=f"psbc{kb}",
                                                 tag="bc")
                                nc.tensor.matmul(
                                    ps_bc[:],
                                    ones_row[:],
                                    qpos_sb[:, qmin:qmin + GQ],
                                    start=True,
                                    stop=True,
                                )
                                qbc = att.tile([P, GQ], F32, name=f"qbc{gmin}",
                                               tag="qbc")
                                nc.vector.tensor_copy(qbc[:], ps_bc[:])
                            ps_s = scp.tile([P, NQ], F32, name=f"pss{kb}",
                                            tag="sc")
                            chunks = [(0, min(512, width))]
                            if width > 512:
                                chunks.append((512, width))
                            for di in range(DT):
                                for a, b2 in chunks:
                                    nc.tensor.matmul(
                                        ps_s[:, a:b2],
                                        kt_sb[:, di, kb * P:(kb + 1) * P],
                                        qt_sb[:, di, qmin + a:qmin + b2],
                                        start=(di == 0),
                                        stop=(di == DT - 1),
                                    )
                            pt = ptp.tile([P, NQ], BF16, name=f"pt{kb}", tag="pt")
                            nc.scalar.activation(
                                pt[:, qmin:NQ], ps_s[:, 0:width], AF.Exp,
                                bias=0.0, scale=1.0 / 32.0,
                            )
                            # causal keep-mask on the diagonal band (one group)
                            keep = att.tile([P, GQ], BF16, name=f"kept{kb}",
                                            tag="keep")
                            if kb == 0:
                                kp = kpos_f
                            else:
                                kp = att.tile([P, 1], F32, name=f"kpt{kb}",
                                              tag="kp")
                                nc.vector.tensor_scalar(
                                    kp[:], kpos_f[:], float(kb * P), None,
                                    mybir.AluOpType.add,
                                )
                            nc.vector.tensor_scalar(
                                keep[:], qbc[:], kp[:], None,
                                mybir.AluOpType.is_ge,
                            )
                            nc.vector.tensor_mul(
                                pt[:, qmin:qmin + GQ], pt[:, qmin:qmin + GQ],
                                keep[:],
                            )
                            pt_tiles.append(pt)

                      # ---- attention pass 2: AV per group, DMA-free ----
                      with tc.tile_pool(name="avp", bufs=6, space="PSUM") as avp, \
                           tc.tile_pool(name="smp", bufs=1, space="PSUM") as smp, \
                           tc.tile_pool(name="evp", bufs=2) as ev:
                        for g in range(NG):
                            lkb = LKB[g]
                            oa = [
                                avp.tile([P, 512], F32, name=f"oa{g}_{i}",
                                         tag="av")
                                for i in range(4)
                            ]
                            sm = [
                                smp.tile([P, 1], F32, name=f"sm{g}_{i}",
                                         tag=f"sm{i}")
                                for i in range(2)
                            ]
                            for kb in range(lkb):
                                v_sb = vcache[kb]
                                first = (kb == 0)
                                last = (kb == lkb - 1)
                                for tq in range(2):
                                    c0 = g * GQ + tq * P
                                    ptq = pt_tiles[kb][:, c0:c0 + P]
                                    nc.tensor.matmul(
                                        oa[2 * tq][:], ptq, v_sb[:, 0:512],
                                        start=first, stop=last,
                                    )
                                    nc.tensor.matmul(
                                        oa[2 * tq + 1][:], ptq,
                                        v_sb[:, 512:1024],
                                        start=first, stop=last,
                                    )
                                    nc.tensor.matmul(
                                        sm[tq][:], ptq, ones_col[:],
                                        start=first, stop=last,
                                    )

                            for tq in range(2):
                                recip = ev.tile([P, 1], F32,
                                                name=f"rc{g}_{tq}", tag="recip")
                                nc.vector.reciprocal(recip[:], sm[tq][:])
                                o_sb = ev.tile([P, D], F32, name=f"ob{g}_{tq}",
                                               tag="osb")
                                nc.scalar.mul(
                                    o_sb[:, 0:512], oa[2 * tq][:], recip[:]
                                )
                                nc.vector.tensor_scalar(
                                    o_sb[:, 512:1024], oa[2 * tq + 1][:],
                                    recip[:], None, mybir.AluOpType.mult,
                                )
                                q_local = 2 * g + tq
                                nc.gpsimd.dma_start(
                                    out=out[q_local * P:(q_local + 1) * P, :],
                                    in_=o_sb[:],
                                )
    nc.compile()
    return nc


def get_nc():
    if not _NC_CACHE:
        _NC_CACHE.append(_build_nc())
    return _NC_CACHE[0]


def _block_w(wT, cols):
    # [D, D] -> [nb, P, KI*cols] with [b, p, ki, o] = wT[ki*P + p, b*cols + o]
    nb = D // cols
    return np.ascontiguousarray(
        wT.reshape(KI, P, nb, cols).transpose(2, 1, 0, 3).reshape(nb, P, KI * cols)
    )


def make_in_maps(x, Wq, Wk, Wv):
    x = np.asarray(x, dtype=np.float32)
    wqb = _block_w(np.asarray(Wq, np.float32).T, P)
    wkb = _block_w(np.asarray(Wk, np.float32).T, P)
    wvb = _block_w(np.asarray(Wv, np.float32).T, 512)
    in_maps = []
    for c in range(8):
        b, h = divmod(c, 2)
        qrows = np.concatenate(
            [np.arange(qb * P, (qb + 1) * P) for qb in QLIST[h]]
        )
        xb = x[b]  # [S, D]
        in_maps.append({
            "xT": np.ascontiguousarray(xb.T),
            "xqT": np.ascontiguousarray(xb[qrows].T),
            "wq": wqb,
            "wk": wkb,
            "wv": wvb,
            "qpos": qrows.astype(np.float32)[None, :],
        })
    return in_maps


def assemble_output(results):
    out = np.empty((B, S, D), dtype=np.float32)
    for c in range(8):
        b, h = divmod(c, 2)
        oc = results[c]["out"]
        for i, qb in enumerate(QLIST[h]):
            out[b, qb * P:(qb + 1) * P, :] = oc[i * P:(i + 1) * P, :]
    return out


def kernel(x, Wq, Wk, Wv):
    nc = get_nc()
    in_maps = make_in_maps(x, Wq, Wk, Wv)
    res = run_bass_kernel_spmd(nc, in_maps, core_ids=list(range(8)), trace=False)
    return assemble_output(res.results)


# revision 23
# speedup vs baseline: 1.2405x; 1.0058x over previous
"""Causal attention (B=4, S=2048, D=1024) on 8 TRN2 NeuronCores.

Sharding: core c -> batch c//2, query-half c%2. Each core computes K/V for
all 2048 keys of its batch and attention for 1024 queries. Queries are
regrouped (host-side) into 4 groups of 256 pairing complementary causal
blocks, so one SPMD program with a fixed key-prefix schedule [4,8,12,16]
kblocks serves both halves; per-core causal structure lives in input data
(xqT column gather + qpos vector), never in program constants.

Math: scoresT[k,q] = KT^T QT accumulated over d in PSUM, probs =
exp(scoresT/32) (no max subtraction: logits ~ N(0,1)), causal mask applied
as a multiplicative (qpos >= kpos) keep-mask after exp, out = P^T V with
row-sums from a ones-column matmul, normalized at eviction.

Precision: Q/K/scores in float32r (tf32-class), V/probs bf16, fp32 accum.

DMA queues are balanced by need-time: sync carries wq0 -> xqT -> wq ->
xT -> wk in arrival order, gpsimd carries wv + v_tmp/out writes, scalar
carries qpos + v_tmp prefetch reads for the AV pass.
"""

from contextlib import ExitStack

import numpy as np

import concourse.bass as bass
import concourse.mybir as mybir
import concourse.tile as tile
from concourse import bacc
from concourse.bass_utils import run_bass_kernel_spmd

B, S, D = 4, 2048, 1024
P = 128
NQ = S // 2               # queries per core
DT = D // P               # 8 d-tiles
KI = D // P               # 8 contraction tiles
NKB = S // P              # 16 key blocks
NG = 4                    # query groups per core
GQ = 256                  # queries per group
LKB = [4, 8, 12, 16]      # key-prefix (in kblocks) per group

# per-core query block order (global block index within the batch)
QLIST = {
    0: [0, 2, 4, 6, 9, 11, 13, 15],
    1: [1, 3, 5, 7, 8, 10, 12, 14],
}

F32 = mybir.dt.float32
F32R = mybir.dt.float32r
BF16 = mybir.dt.bfloat16
AF = mybir.ActivationFunctionType

_NC_CACHE = []


def _build_nc():
    nc = bacc.Bacc("TRN2")
    xT = nc.dram_tensor("xT", [D, S], F32, kind="ExternalInput")
    xqT = nc.dram_tensor("xqT", [D, NQ], F32, kind="ExternalInput")
    # weights host-blocked: [DT, P, KI*P] with [dt, p, ki, o] = WT[ki*P+p, dt*P+o]
    wq = nc.dram_tensor("wq", [DT, P, KI * P], F32, kind="ExternalInput")
    wk = nc.dram_tensor("wk", [DT, P, KI * P], F32, kind="ExternalInput")
    # wv blocked: [2, P, KI*512] with [oc, p, ki, o] = WvT[ki*P+p, oc*512+o]
    wv = nc.dram_tensor("wv", [2, P, KI * 512], F32, kind="ExternalInput")
    qpos = nc.dram_tensor("qpos", [1, NQ], F32, kind="ExternalInput")
    out = nc.dram_tensor("out", [NQ, D], F32, kind="ExternalOutput")
    v_tmp = nc.dram_tensor("v_tmp", [NKB, P, D], BF16, kind="Internal")

    with tile.TileContext(nc) as tc:
        with tc.tile_pool(name="const", bufs=1) as const:
            ones_row = const.tile([1, P], F32, name="ones_row")
            nc.vector.memset(ones_row[:], 1.0)
            ones_col = const.tile([P, 1], BF16, name="ones_col")
            nc.vector.memset(ones_col[:], 1.0)
            kpos_i = const.tile([P, 1], mybir.dt.int32, name="kpos_i")
            nc.gpsimd.iota(kpos_i[:], pattern=[[0, 1]], base=0, channel_multiplier=1)
            kpos_f = const.tile([P, 1], F32, name="kpos_f")
            nc.vector.tensor_copy(kpos_f[:], kpos_i[:])
            qpos_sb = const.tile([1, NQ], F32, name="qpos_sb")
            nc.scalar.dma_start(out=qpos_sb[:], in_=qpos[:])

            # PE warmup: throwaway matmuls during the initial DMA wait so the
            # HAM clock gate opens before real work arrives.
            with tc.tile_pool(name="warm", bufs=1) as warm, \
                 tc.tile_pool(name="warmp", bufs=1, space="PSUM") as warmp:
                wsink = warmp.tile([P, 512], F32, name="wsink")
                wzero = warm.tile([1, 512], F32, name="wzero")
                nc.vector.memset(wzero[:], 0.0)
                for i in range(10):
                    nc.tensor.matmul(wsink[:], ones_row[:], wzero[:],
                                     start=True, stop=True)
                wdrain = warm.tile([P, 1], F32, name="wdrain")
                nc.scalar.copy(wdrain[:], wsink[:, 0:1])

            with tc.tile_pool(name="qt_p", bufs=1) as qt_pool:
                qt_sb = qt_pool.tile([P, DT, NQ], F32R, name="qt_sb")
                xtr_stack = ExitStack()
                xtr_pool = xtr_stack.enter_context(
                    tc.tile_pool(name="xtr_p", bufs=8, side="right")
                )
                xtr = []

                # ---- Q projection ----
                # sync-queue emission order = arrival order: wq0, then xq
                # (interleaved with wq1..3), then remaining wq, then xT, wk.
                with tc.tile_pool(name="xqr_p", bufs=8) as xqr_pool, \
                     tc.tile_pool(name="xq_st", bufs=2) as xqstage, \
                     tc.tile_pool(name="x_st", bufs=2) as xstage, \
                     tc.tile_pool(name="wq_s", bufs=3) as wq_stage, \
                     tc.tile_pool(name="wq_r", bufs=3) as wq_round, \
                     tc.tile_pool(name="pj_q", bufs=8, space="PSUM") as pj:
                    wq_rs = {}

                    def load_wq(dt):
                        stw = wq_stage.tile([P, KI * P], F32, name=f"wqs{dt}",
                                            tag="wqs")
                        nc.sync.dma_start(out=stw[:], in_=wq[dt, :, :])
                        wq_rs[dt] = wq_round.tile([P, KI * P], F32R,
                                                  name=f"wqr{dt}", tag="wqr")
                        nc.vector.tensor_copy(wq_rs[dt][:], stw[:])

                    load_wq(0)
                    xqr = []
                    for ki in range(KI):
                        stg = xqstage.tile([P, NQ], F32, name=f"xqs{ki}", tag="xqs")
                        eng = nc.sync if ki % 2 == 0 else nc.scalar
                        eng.dma_start(
                            out=stg[:], in_=xqT[ki * P:(ki + 1) * P, :]
                        )
                        t = xqr_pool.tile([P, NQ], F32R, name=f"xqr{ki}", tag="xqr")
                        nc.vector.tensor_copy(t[:], stg[:])
                        xqr.append(t)
                        if ki in (2, 4, 6):
                            load_wq(ki // 2)
                    for dt in range(DT):
                        if dt + 4 < DT:
                            load_wq(dt + 4)
                        wq_r = wq_rs[dt]
                        # one xT column-block load per dt iteration
                        stg = xstage.tile([P, S], F32, name=f"xs{dt}", tag="xs")
                        nc.sync.dma_start(
                            out=stg[:], in_=xT[dt * P:(dt + 1) * P, :]
                        )
                        t = xtr_pool.tile([P, S], F32R, name=f"xtr{dt}", tag="xtr")
                        nc.vector.tensor_copy(t[:], stg[:])
                        xtr.append(t)
                        ps = [
                            pj.tile([P, 512], F32, name=f"psq{dt}_{qc}", tag="pjq")
                            for qc in range(2)
                        ]
                        for ki in range(KI):
                            for qc in range(2):
                                nc.tensor.matmul(
                                    ps[qc][:],
                                    wq_r[:, ki * P:(ki + 1) * P],
                                    xqr[ki][:, qc * 512:(qc + 1) * 512],
                                    start=(ki == 0),
                                    stop=(ki == KI - 1),
                                )
                        for qc in range(2):
                            nc.scalar.copy(
                                qt_sb[:, dt, qc * 512:(qc + 1) * 512], ps[qc][:]
                            )

                # ---- K projection ----
                with tc.tile_pool(name="kt_p", bufs=1) as kt_pool:
                    kt_sb = kt_pool.tile([P, DT, S], F32R, name="kt_sb")
                    with tc.tile_pool(name="wk_s", bufs=3) as wk_stage, \
                         tc.tile_pool(name="wk_r", bufs=2) as wk_round, \
                         tc.tile_pool(name="pj_k", bufs=8, space="PSUM") as pjk:
                        for dt in range(DT):
                            wk_st = wk_stage.tile([P, KI * P], F32, name=f"wks{dt}",
                                                  tag="wks")
                            nc.gpsimd.dma_start(out=wk_st[:], in_=wk[dt, :, :])
                            wk_r = wk_round.tile([P, KI * P], F32R, name=f"wkr{dt}",
                                                 tag="wkr")
                            nc.vector.tensor_copy(wk_r[:], wk_st[:])
                            ps = [
                                pjk.tile([P, 512], F32, name=f"psk{dt}_{kc}",
                                         tag="pjk")
                                for kc in range(4)
                            ]
                            for ki in range(KI):
                                for kc in range(4):
                                    nc.tensor.matmul(
                                        ps[kc][:],
                                        wk_r[:, ki * P:(ki + 1) * P],
                                        xtr[ki][:, kc * 512:(kc + 1) * 512],
                                        start=(ki == 0),
                                        stop=(ki == KI - 1),
                                    )
                            for kc in range(4):
                                nc.scalar.copy(
                                    kt_sb[:, dt, kc * 512:(kc + 1) * 512], ps[kc][:]
                                )

                    # ---- V projection -> DRAM (bf16) ----
                    with tc.tile_pool(name="wv_s", bufs=2) as wv_stage, \
                         tc.tile_pool(name="wv_r", bufs=8) as wv_round, \
                         tc.tile_pool(name="v_ev", bufs=3) as vst_pool, \
                         tc.tile_pool(name="pj_v", bufs=8, space="PSUM") as pjv:
                        for oc in range(2):
                            wv_r = []
                            for ki in range(KI):
                                stg = wv_stage.tile([P, 512], F32,
                                                    name=f"wvs{oc}_{ki}", tag="wvs")
                                nc.gpsimd.dma_start(
                                    out=stg[:],
                                    in_=wv[oc, :, ki * 512:(ki + 1) * 512],
                                )
                                t = wv_round.tile([P, 512], F32R,
                                                  name=f"wvr{oc}_{ki}", tag="wvr")
                                nc.vector.tensor_copy(t[:], stg[:])
                                wv_r.append(t)
                            for st_i in range(NKB):
                                ps = pjv.tile([P, 512], F32, name=f"psv{oc}_{st_i}",
                                              tag="pjv")
                                for ki in range(KI):
                                    nc.tensor.matmul(
                                        ps[:],
                                        xtr[ki][:, st_i * P:(st_i + 1) * P],
                                        wv_r[ki][:],
                                        start=(ki == 0),
                                        stop=(ki == KI - 1),
                                    )
                                vs = vst_pool.tile([P, 512], BF16,
                                                   name=f"vsb{oc}_{st_i}", tag="vsb")
                                nc.scalar.copy(vs[:], ps[:])
                                nc.gpsimd.dma_start(
                                    out=v_tmp[st_i, :, oc * 512:(oc + 1) * 512],
                                    in_=vs[:],
                                )

                    xtr_stack.close()  # free xT (64KB/p) before attention

                    # ---- attention pass 1: all probs tiles, kb-major ----
                    # scoresT for kblock kb covers queries [qmin:1024] where
                    # qmin = (kb//4)*256 (groups below are causally done).
                    # V blocks prefetch into SBUF so the AV pass is DMA-free.
                    pt_tiles = []
                    vcache = []
                    with tc.tile_pool(name="ptp", bufs=16) as ptp, \
                         tc.tile_pool(name="vcp", bufs=16) as vcp:
                      with tc.tile_pool(name="att", bufs=2) as att, \
                           tc.tile_pool(name="scp", bufs=3, space="PSUM") as scp, \
                           tc.tile_pool(name="bcp", bufs=2, space="PSUM") as bcp:
                        qbc = None
                        for kb in range(NKB):
                            v_sb = vcp.tile([P, D], BF16, name=f"vc{kb}", tag="vc")
                            nc.scalar.dma_start(out=v_sb[:], in_=v_tmp[kb, :, :])
                            vcache.append(v_sb)

                            gmin = kb // 4
                            qmin = gmin * GQ
                            width = NQ - qmin
                            if kb % 4 == 0:
                                ps_bc = bcp.tile([P, GQ], F32, name=f"psbc{kb}",
                                                 tag="bc")
                                nc.tensor.matmul(
                                    ps_bc[:],
                                    ones_row[:],
                                    qpos_sb[:, qmin:qmin + GQ],
                                    start=True,
                                    stop=True,
                                )
                                qbc = att.tile([P, GQ], F32, name=f"qbc{gmin}",
                                               tag="qbc")
                                nc.vector.tensor_copy(qbc[:], ps_bc[:])
                            ps_s = scp.tile([P, NQ], F32, name=f"pss{kb}",
                                            tag="sc")
                            chunks = [(0, min(512, width))]
                            if width > 512:
                                chunks.append((512, width))
                            for di in range(DT):
                                for a, b2 in chunks:
                                    nc.tensor.matmul(
                                        ps_s[:, a:b2],
                                        kt_sb[:, di, kb * P:(kb + 1) * P],
                                        qt_sb[:, di, qmin + a:qmin + b2],
                                        start=(di == 0),
                                        stop=(di == DT - 1),
                                    )
                            pt = ptp.tile([P, NQ], BF16, name=f"pt{kb}", tag="pt")
                            nc.scalar.activation(
                                pt[:, qmin:NQ], ps_s[:, 0:width], AF.Exp,
                                bias=0.0, scale=1.0 / 32.0,
                            )
                            # causal keep-mask on the diagonal band (one group)
                            keep = att.tile([P, GQ], BF16, name=f"kept{kb}",
                                            tag="keep")
                            if kb == 0:
                                kp = kpos_f
                            else:
                                kp = att.tile([P, 1], F32, name=f"kpt{kb}",
                                              tag="kp")
                                nc.vector.tensor_scalar(
                                    kp[:], kpos_f[:], float(kb * P), None,
                                    mybir.AluOpType.add,
                                )
                            nc.vector.tensor_scalar(
                                keep[:], qbc[:], kp[:], None,
                                mybir.AluOpType.is_ge,
                            )
                            nc.vector.tensor_mul(
                                pt[:, qmin:qmin + GQ], pt[:, qmin:qmin + GQ],
                                keep[:],
                            )
                            pt_tiles.append(pt)

                      # ---- attention pass 2: AV per group, DMA-free ----
                      with tc.tile_pool(name="avp", bufs=6, space="PSUM") as avp, \
                           tc.tile_pool(name="smp", bufs=1, space="PSUM") as smp, \
                           tc.tile_pool(name="evp", bufs=2) as ev:
                        for g in range(NG):
                            lkb = LKB[g]
                            oa = [
                                avp.tile([P, 512], F32, name=f"oa{g}_{i}",
                                         tag="av")
                                for i in range(4)
                            ]
                            sm = [
                                smp.tile([P, 1], F32, name=f"sm{g}_{i}",
                                         tag=f"sm{i}")
                                for i in range(2)
                            ]
                            for kb in range(lkb):
                                v_sb = vcache[kb]
                                first = (kb == 0)
                                last = (kb == lkb - 1)
                                for tq in range(2):
                                    c0 = g * GQ + tq * P
                                    ptq = pt_tiles[kb][:, c0:c0 + P]
                                    nc.tensor.matmul(
                                        oa[2 * tq][:], ptq, v_sb[:, 0:512],
                                        start=first, stop=last,
                                    )
                                    nc.tensor.matmul(
                                        oa[2 * tq + 1][:], ptq,
                                        v_sb[:, 512:1024],
                                        start=first, stop=last,
                                    )
                                    nc.tensor.matmul(
                                        sm[tq][:], ptq, ones_col[:],
                                        start=first, stop=last,
                                    )

                            for tq in range(2):
                                recip = ev.tile([P, 1], F32,
                                                name=f"rc{g}_{tq}", tag="recip")
                                nc.vector.reciprocal(recip[:], sm[tq][:])
                                o_sb = ev.tile([P, D], F32, name=f"ob{g}_{tq}",
                                               tag="osb")
                                nc.scalar.mul(
                                    o_sb[:, 0:512], oa[2 * tq][:], recip[:]
                                )
                                nc.vector.tensor_scalar(
                                    o_sb[:, 512:1024], oa[2 * tq + 1][:],
                                    recip[:], None, mybir.AluOpType.mult,
                                )
                                q_local = 2 * g + tq
                                nc.gpsimd.dma_start(
                                    out=out[q_local * P:(q_local + 1) * P, :],
                                    in_=o_sb[:],
                                )
    nc.compile()
    return nc


def get_nc():
    if not _NC_CACHE:
        _NC_CACHE.append(_build_nc())
    return _NC_CACHE[0]


def _block_w(wT, cols):
    # [D, D] -> [nb, P, KI*cols] with [b, p, ki, o] = wT[ki*P + p, b*cols + o]
    nb = D // cols
    return np.ascontiguousarray(
        wT.reshape(KI, P, nb, cols).transpose(2, 1, 0, 3).reshape(nb, P, KI * cols)
    )


def make_in_maps(x, Wq, Wk, Wv):
    x = np.asarray(x, dtype=np.float32)
    wqb = _block_w(np.asarray(Wq, np.float32).T, P)
    wkb = _block_w(np.asarray(Wk, np.float32).T, P)
    wvb = _block_w(np.asarray(Wv, np.float32).T, 512)
    in_maps = []
    for c in range(8):
        b, h = divmod(c, 2)
        qrows = np.concatenate(
            [np.arange(qb * P, (qb + 1) * P) for qb in QLIST[h]]
        )
        xb = x[b]  # [S, D]
        in_maps.append({
            "xT": np.ascontiguousarray(xb.T),
            "xqT": np.ascontiguousarray(xb[qrows].T),
            "wq": wqb,
            "wk": wkb,
            "wv": wvb,
            "qpos": qrows.astype(np.float32)[None, :],
        })
    return in_maps


def assemble_output(results):
    out = np.empty((B, S, D), dtype=np.float32)
    for c in range(8):
        b, h = divmod(c, 2)
        oc = results[c]["out"]
        for i, qb in enumerate(QLIST[h]):
            out[b, qb * P:(qb + 1) * P, :] = oc[i * P:(i + 1) * P, :]
    return out


def kernel(x, Wq, Wk, Wv):
    nc = get_nc()
    in_maps = make_in_maps(x, Wq, Wk, Wv)
    res = run_bass_kernel_spmd(nc, in_maps, core_ids=list(range(8)), trace=False)
    return assemble_output(res.results)


# revision 25
# speedup vs baseline: 1.2562x; 1.0126x over previous
"""Causal attention (B=4, S=2048, D=1024) on 8 TRN2 NeuronCores.

Sharding: core c -> batch c//2, query-half c%2. Each core computes K/V for
all 2048 keys of its batch and attention for 1024 queries. Queries are
regrouped (host-side) into 4 groups of 256 pairing complementary causal
blocks, so one SPMD program with a fixed key-prefix schedule [4,8,12,16]
kblocks serves both halves; per-core causal structure lives in input data
(xqT column gather + qpos vector), never in program constants.

Math: scoresT[k,q] = KT^T QT accumulated over d in PSUM, probs =
exp(scoresT/32) (no max subtraction: logits ~ N(0,1)), causal mask applied
as a multiplicative (qpos >= kpos) keep-mask after exp, out = P^T V with
row-sums from a ones-column matmul, normalized at eviction.

Precision: Q/K/scores in float32r (tf32-class), V/probs bf16, fp32 accum.

DMA queues are balanced by need-time: sync carries wq0 -> xqT -> wq ->
xT -> wk in arrival order, gpsimd carries wv + v_tmp/out writes, scalar
carries qpos + v_tmp prefetch reads for the AV pass.
"""

from contextlib import ExitStack

import numpy as np

import concourse.bass as bass
import concourse.mybir as mybir
import concourse.tile as tile
from concourse import bacc
from concourse.bass_utils import run_bass_kernel_spmd

B, S, D = 4, 2048, 1024
P = 128
NQ = S // 2               # queries per core
DT = D // P               # 8 d-tiles
KI = D // P               # 8 contraction tiles
NKB = S // P              # 16 key blocks
NG = 4                    # query groups per core
GQ = 256                  # queries per group
LKB = [4, 8, 12, 16]      # key-prefix (in kblocks) per group

# per-core query block order (global block index within the batch)
QLIST = {
    0: [0, 2, 4, 6, 9, 11, 13, 15],
    1: [1, 3, 5, 7, 8, 10, 12, 14],
}

F32 = mybir.dt.float32
F32R = mybir.dt.float32r
BF16 = mybir.dt.bfloat16
AF = mybir.ActivationFunctionType

_NC_CACHE = []


def _build_nc():
    nc = bacc.Bacc("TRN2")
    xT = nc.dram_tensor("xT", [D, S], F32, kind="ExternalInput")
    xqT = nc.dram_tensor("xqT", [D, NQ], F32, kind="ExternalInput")
    # weights host-blocked: [DT, P, KI*P] with [dt, p, ki, o] = WT[ki*P+p, dt*P+o]
    wq = nc.dram_tensor("wq", [DT, P, KI * P], F32, kind="ExternalInput")
    wk = nc.dram_tensor("wk", [DT, P, KI * P], F32, kind="ExternalInput")
    # wv blocked: [2, P, KI*512] with [oc, p, ki, o] = WvT[ki*P+p, oc*512+o]
    wv = nc.dram_tensor("wv", [2, P, KI * 512], F32, kind="ExternalInput")
    qpos = nc.dram_tensor("qpos", [1, NQ], F32, kind="ExternalInput")
    out = nc.dram_tensor("out", [NQ, D], F32, kind="ExternalOutput")
    v_tmp = nc.dram_tensor("v_tmp", [NKB, P, D], BF16, kind="Internal")

    with tile.TileContext(nc) as tc:
        with tc.tile_pool(name="const", bufs=1) as const:
            ones_row = const.tile([1, P], F32, name="ones_row")
            nc.vector.memset(ones_row[:], 1.0)

            # PE warmup first: throwaway matmuls during the initial DMA wait
            # so the HAM clock gate opens before real work arrives. Emitted
            # before the iota/kpos chain so PE's dep is DVE's first op.
            with tc.tile_pool(name="warm", bufs=1) as warm, \
                 tc.tile_pool(name="warmp", bufs=1, space="PSUM") as warmp:
                wsink = warmp.tile([P, 512], F32, name="wsink")
                wzero = warm.tile([1, 512], F32, name="wzero")
                nc.vector.memset(wzero[:], 0.0)
                for i in range(10):
                    nc.tensor.matmul(wsink[:], ones_row[:], wzero[:],
                                     start=True, stop=True)
                wdrain = warm.tile([P, 1], F32, name="wdrain")
                nc.scalar.copy(wdrain[:], wsink[:, 0:1])

            ones_col = const.tile([P, 1], BF16, name="ones_col")
            nc.vector.memset(ones_col[:], 1.0)
            kpos_i = const.tile([P, 1], mybir.dt.int32, name="kpos_i")
            nc.gpsimd.iota(kpos_i[:], pattern=[[0, 1]], base=0, channel_multiplier=1)
            kpos_f = const.tile([P, 1], F32, name="kpos_f")
            nc.vector.tensor_copy(kpos_f[:], kpos_i[:])
            qpos_sb = const.tile([1, NQ], F32, name="qpos_sb")
            nc.scalar.dma_start(out=qpos_sb[:], in_=qpos[:])

            # wk0 staged through never-aliased const space: its DMA has no
            # slot-release dependency, so it prefetches at t=0 and K proj
            # starts the moment Q's PE work drains.
            wk0_st = const.tile([P, KI * P], F32, name="wk0_st")
            nc.gpsimd.dma_start(out=wk0_st[:], in_=wk[0, :, :])
            wk0_r = const.tile([P, KI * P], F32R, name="wk0_r")
            nc.vector.tensor_copy(wk0_r[:], wk0_st[:])

            with tc.tile_pool(name="qt_p", bufs=1) as qt_pool:
                qt_sb = qt_pool.tile([P, DT, NQ], F32R, name="qt_sb")
                xtr_stack = ExitStack()
                xtr_pool = xtr_stack.enter_context(
                    tc.tile_pool(name="xtr_p", bufs=8, side="right")
                )
                xtr = []

                # ---- Q projection ----
                # sync-queue emission order = arrival order: wq0, then xq
                # (interleaved with wq1..3), then remaining wq, then xT, wk.
                with tc.tile_pool(name="xqr_p", bufs=8) as xqr_pool, \
                     tc.tile_pool(name="xq_st", bufs=2) as xqstage, \
                     tc.tile_pool(name="x_st", bufs=2) as xstage, \
                     tc.tile_pool(name="wq_s", bufs=3) as wq_stage, \
                     tc.tile_pool(name="wq_r", bufs=2) as wq_round, \
                     tc.tile_pool(name="pj_q", bufs=8, space="PSUM") as pj:
                    wq_rs = {}

                    def load_wq(dt):
                        stw = wq_stage.tile([P, KI * P], F32, name=f"wqs{dt}",
                                            tag="wqs")
                        nc.sync.dma_start(out=stw[:], in_=wq[dt, :, :])
                        wq_rs[dt] = wq_round.tile([P, KI * P], F32R,
                                                  name=f"wqr{dt}", tag="wqr")
                        nc.vector.tensor_copy(wq_rs[dt][:], stw[:])

                    load_wq(0)
                    xqr = []
                    for ki in range(KI):
                        stg = xqstage.tile([P, NQ], F32, name=f"xqs{ki}", tag="xqs")
                        eng = nc.sync if ki % 2 == 0 else nc.scalar
                        eng.dma_start(
                            out=stg[:], in_=xqT[ki * P:(ki + 1) * P, :]
                        )
                        t = xqr_pool.tile([P, NQ], F32R, name=f"xqr{ki}", tag="xqr")
                        nc.vector.tensor_copy(t[:], stg[:])
                        xqr.append(t)
                        if ki in (2, 4, 6):
                            load_wq(ki // 2)
                    for dt in range(DT):
                        if dt + 4 < DT:
                            load_wq(dt + 4)
                        wq_r = wq_rs[dt]
                        # one xT column-block load per dt iteration
                        stg = xstage.tile([P, S], F32, name=f"xs{dt}", tag="xs")
                        nc.sync.dma_start(
                            out=stg[:], in_=xT[dt * P:(dt + 1) * P, :]
                        )
                        t = xtr_pool.tile([P, S], F32R, name=f"xtr{dt}", tag="xtr")
                        nc.vector.tensor_copy(t[:], stg[:])
                        xtr.append(t)
                        ps = [
                            pj.tile([P, 512], F32, name=f"psq{dt}_{qc}", tag="pjq")
                            for qc in range(2)
                        ]
                        for ki in range(KI):
                            for qc in range(2):
                                nc.tensor.matmul(
                                    ps[qc][:],
                                    wq_r[:, ki * P:(ki + 1) * P],
                                    xqr[ki][:, qc * 512:(qc + 1) * 512],
                                    start=(ki == 0),
                                    stop=(ki == KI - 1),
                                )
                        for qc in range(2):
                            nc.scalar.copy(
                                qt_sb[:, dt, qc * 512:(qc + 1) * 512], ps[qc][:]
                            )

                # ---- K projection ----
                with tc.tile_pool(name="kt_p", bufs=1) as kt_pool:
                    kt_sb = kt_pool.tile([P, DT, S], F32R, name="kt_sb")
                    with tc.tile_pool(name="wk_s", bufs=3) as wk_stage, \
                         tc.tile_pool(name="wk_r", bufs=2) as wk_round, \
                         tc.tile_pool(name="pj_k", bufs=8, space="PSUM") as pjk:
                        for dt in range(DT):
                            if dt == 0:
                                wk_r = wk0_r
                            else:
                                wk_st = wk_stage.tile([P, KI * P], F32,
                                                      name=f"wks{dt}", tag="wks")
                                nc.gpsimd.dma_start(out=wk_st[:], in_=wk[dt, :, :])
                                wk_r = wk_round.tile([P, KI * P], F32R,
                                                     name=f"wkr{dt}", tag="wkr")
                                nc.vector.tensor_copy(wk_r[:], wk_st[:])
                            ps = [
                                pjk.tile([P, 512], F32, name=f"psk{dt}_{kc}",
                                         tag="pjk")
                                for kc in range(4)
                            ]
                            for ki in range(KI):
                                for kc in range(4):
                                    nc.tensor.matmul(
                                        ps[kc][:],
                                        wk_r[:, ki * P:(ki + 1) * P],
                                        xtr[ki][:, kc * 512:(kc + 1) * 512],
                                        start=(ki == 0),
                                        stop=(ki == KI - 1),
                                    )
                            for kc in range(4):
                                nc.scalar.copy(
                                    kt_sb[:, dt, kc * 512:(kc + 1) * 512], ps[kc][:]
                                )

                    # ---- V projection -> DRAM (bf16) ----
                    with tc.tile_pool(name="wv_s", bufs=2) as wv_stage, \
                         tc.tile_pool(name="wv_r", bufs=8) as wv_round, \
                         tc.tile_pool(name="v_ev", bufs=3) as vst_pool, \
                         tc.tile_pool(name="pj_v", bufs=8, space="PSUM") as pjv:
                        for oc in range(2):
                            wv_r = []
                            for ki in range(KI):
                                stg = wv_stage.tile([P, 512], F32,
                                                    name=f"wvs{oc}_{ki}", tag="wvs")
                                nc.gpsimd.dma_start(
                                    out=stg[:],
                                    in_=wv[oc, :, ki * 512:(ki + 1) * 512],
                                )
                                t = wv_round.tile([P, 512], F32R,
                                                  name=f"wvr{oc}_{ki}", tag="wvr")
                                nc.vector.tensor_copy(t[:], stg[:])
                                wv_r.append(t)
                            for st_i in range(NKB):
                                ps = pjv.tile([P, 512], F32, name=f"psv{oc}_{st_i}",
                                              tag="pjv")
                                for ki in range(KI):
                                    nc.tensor.matmul(
                                        ps[:],
                                        xtr[ki][:, st_i * P:(st_i + 1) * P],
                                        wv_r[ki][:],
                                        start=(ki == 0),
                                        stop=(ki == KI - 1),
                                    )
                                vs = vst_pool.tile([P, 512], BF16,
                                                   name=f"vsb{oc}_{st_i}", tag="vsb")
                                nc.scalar.copy(vs[:], ps[:])
                                nc.gpsimd.dma_start(
                                    out=v_tmp[st_i, :, oc * 512:(oc + 1) * 512],
                                    in_=vs[:],
                                )

                    xtr_stack.close()  # free xT (64KB/p) before attention

                    # ---- attention pass 1: all probs tiles, kb-major ----
                    # scoresT for kblock kb covers queries [qmin:1024] where
                    # qmin = (kb//4)*256 (groups below are causally done).
                    # V blocks prefetch into SBUF so the AV pass is DMA-free.
                    pt_tiles = []
                    vcache = []
                    with tc.tile_pool(name="ptp", bufs=16) as ptp, \
                         tc.tile_pool(name="vcp", bufs=16) as vcp:
                      with tc.tile_pool(name="att", bufs=2) as att, \
                           tc.tile_pool(name="scp", bufs=3, space="PSUM") as scp, \
                           tc.tile_pool(name="bcp", bufs=2, space="PSUM") as bcp:
                        qbc = None
                        for kb in range(NKB):
                            v_sb = vcp.tile([P, D], BF16, name=f"vc{kb}", tag="vc")
                            nc.scalar.dma_start(out=v_sb[:], in_=v_tmp[kb, :, :])
                            vcache.append(v_sb)

                            gmin = kb // 4
                            qmin = gmin * GQ
                            width = NQ - qmin
                            if kb % 4 == 0:
                                ps_bc = bcp.tile([P, GQ], F32, name=f"psbc{kb}",
                                                 tag="bc")
                                nc.tensor.matmul(
                                    ps_bc[:],
                                    ones_row[:],
                                    qpos_sb[:, qmin:qmin + GQ],
                                    start=True,
                                    stop=True,
                                )
                                qbc = att.tile([P, GQ], F32, name=f"qbc{gmin}",
                                               tag="qbc")
                                nc.vector.tensor_copy(qbc[:], ps_bc[:])
                            ps_s = scp.tile([P, NQ], F32, name=f"pss{kb}",
                                            tag="sc")
                            chunks = [(0, min(512, width))]
                            if width > 512:
                                chunks.append((512, width))
                            for di in range(DT):
                                for a, b2 in chunks:
                                    nc.tensor.matmul(
                                        ps_s[:, a:b2],
                                        kt_sb[:, di, kb * P:(kb + 1) * P],
                                        qt_sb[:, di, qmin + a:qmin + b2],
                                        start=(di == 0),
                                        stop=(di == DT - 1),
                                    )
                            pt = ptp.tile([P, NQ], BF16, name=f"pt{kb}", tag="pt")
                            nc.scalar.activation(
                                pt[:, qmin:NQ], ps_s[:, 0:width], AF.Exp,
                                bias=0.0, scale=1.0 / 32.0,
                            )
                            # causal keep-mask on the diagonal band (one group)
                            keep = att.tile([P, GQ], BF16, name=f"kept{kb}",
                                            tag="keep")
                            if kb == 0:
                                kp = kpos_f
                            else:
                                kp = att.tile([P, 1], F32, name=f"kpt{kb}",
                                              tag="kp")
                                nc.vector.tensor_scalar(
                                    kp[:], kpos_f[:], float(kb * P), None,
                                    mybir.AluOpType.add,
                                )
                            nc.vector.tensor_scalar(
                                keep[:], qbc[:], kp[:], None,
                                mybir.AluOpType.is_ge,
                            )
                            nc.vector.tensor_mul(
                                pt[:, qmin:qmin + GQ], pt[:, qmin:qmin + GQ],
                                keep[:],
                            )
                            pt_tiles.append(pt)

                      # ---- attention pass 2: AV per group, DMA-free ----
                      with tc.tile_pool(name="avp", bufs=6, space="PSUM") as avp, \
                           tc.tile_pool(name="smp", bufs=1, space="PSUM") as smp, \
                           tc.tile_pool(name="evp", bufs=2) as ev:
                        for g in range(NG):
                            lkb = LKB[g]
                            oa = [
                                avp.tile([P, 512], F32, name=f"oa{g}_{i}",
                                         tag="av")
                                for i in range(4)
                            ]
                            sm = [
                                smp.tile([P, 1], F32, name=f"sm{g}_{i}",
                                         tag=f"sm{i}")
                                for i in range(2)
                            ]
                            for kb in range(lkb):
                                v_sb = vcache[kb]
                                first = (kb == 0)
                                last = (kb == lkb - 1)
                                for tq in range(2):
                                    c0 = g * GQ + tq * P
                                    ptq = pt_tiles[kb][:, c0:c0 + P]
                                    nc.tensor.matmul(
                                        oa[2 * tq][:], ptq, v_sb[:, 0:512],
                                        start=first, stop=last,
                                    )
                                    nc.tensor.matmul(
                                        oa[2 * tq + 1][:], ptq,
                                        v_sb[:, 512:1024],
                                        start=first, stop=last,
                                    )
                                    nc.tensor.matmul(
                                        sm[tq][:], ptq, ones_col[:],
                                        start=first, stop=last,
                                    )

                            for tq in range(2):
                                recip = ev.tile([P, 1], F32,
                                                name=f"rc{g}_{tq}", tag="recip")
                                nc.vector.reciprocal(recip[:], sm[tq][:])
                                o_sb = ev.tile([P, D], F32, name=f"ob{g}_{tq}",
                                               tag="osb")
                                nc.scalar.mul(
                                    o_sb[:, 0:512], oa[2 * tq][:], recip[:]
                                )
                                nc.vector.tensor_scalar(
                                    o_sb[:, 512:1024], oa[2 * tq + 1][:],
                                    recip[:], None, mybir.AluOpType.mult,
                                )
                                q_local = 2 * g + tq
                                nc.gpsimd.dma_start(
                                    out=out[q_local * P:(q_local + 1) * P, :],
                                    in_=o_sb[:],
                                )
    nc.compile()
    return nc


def get_nc():
    if not _NC_CACHE:
        _NC_CACHE.append(_build_nc())
    return _NC_CACHE[0]


def _block_w(wT, cols):
    # [D, D] -> [nb, P, KI*cols] with [b, p, ki, o] = wT[ki*P + p, b*cols + o]
    nb = D // cols
    return np.ascontiguousarray(
        wT.reshape(KI, P, nb, cols).transpose(2, 1, 0, 3).reshape(nb, P, KI * cols)
    )


def make_in_maps(x, Wq, Wk, Wv):
    x = np.asarray(x, dtype=np.float32)
    wqb = _block_w(np.asarray(Wq, np.float32).T, P)
    wkb = _block_w(np.asarray(Wk, np.float32).T, P)
    wvb = _block_w(np.asarray(Wv, np.float32).T, 512)
    in_maps = []
    for c in range(8):
        b, h = divmod(c, 2)
        qrows = np.concatenate(
            [np.arange(qb * P, (qb + 1) * P) for qb in QLIST[h]]
        )
        xb = x[b]  # [S, D]
        in_maps.append({
            "xT": np.ascontiguousarray(xb.T),
            "xqT": np.ascontiguousarray(xb[qrows].T),
            "wq": wqb,
            "wk": wkb,
            "wv": wvb,
            "qpos": qrows.astype(np.float32)[None, :],
        })
    return in_maps


def assemble_output(results):
    out = np.empty((B, S, D), dtype=np.float32)
    for c in range(8):
        b, h = divmod(c, 2)
        oc = results[c]["out"]
        for i, qb in enumerate(QLIST[h]):
            out[b, qb * P:(qb + 1) * P, :] = oc[i * P:(i + 1) * P, :]
    return out


def kernel(x, Wq, Wk, Wv):
    nc = get_nc()
    in_maps = make_in_maps(x, Wq, Wk, Wv)
    res = run_bass_kernel_spmd(nc, in_maps, core_ids=list(range(8)), trace=False)
    return assemble_output(res.results)
